# revision 7
# baseline (speedup 1.0000x reference)
"""Transformer block (dense_transformer) on 8 TRN2 NeuronCores.

Strategy: pure data-parallel over batch (B=128 -> 16 items/core), weights
replicated. Per item, all linear layers run feature-major ([feat, T] with
feat on partitions); LayerNorm/softmax run token-major ([T, feat]).
Matmul datapath is bf16 (2x PE stream rate vs f32r); residual stream and
softmax statistics stay f32. The softmax normalize is fused into the
weight transpose on PE via a diagonal 1/rowsum matrix built on GpSimd.
LN stats for group g+1 are computed during group g so PE never idles at
group boundaries.
"""

import numpy as np
import ml_dtypes

import concourse.bass as bass
import concourse.mybir as mybir
from concourse.tile import TileContext
from concourse.vector_clock import ScopedClock

F32 = mybir.dt.float32
BF16 = mybir.dt.bfloat16
AF = mybir.ActivationFunctionType
AX = mybir.AxisListType
ALU = mybir.AluOpType

B, T, C, H, D = 128, 256, 384, 6, 64
F = 4 * C
NCORES = 8
BL = B // NCORES
P = 128
TT = T // P    # 2 token tiles
CT = C // P    # 3 channel tiles
FT = F // P    # 12 ffn-hidden tiles
H2 = H // 2    # head pairs
LN_EPS = 1e-5
CSCALE = float(C) ** -0.5
NEG = -1.0e9


class PatchedTileContext(TileContext):
    """Workaround for this container's walrus: BIR instructions may carry at
    most ONE attached sem wait. Hoist extras into standalone waits."""

    def _hoist_multi_waits(self):
        nc = self.nc
        assert self.sems is not None
        sem_by_num = {s.num: s for s in self.sems.allocated().values()}
        for func in nc.m.functions:
            for blk in func.blocks:
                insts = blk.instructions
                i = 0
                while i < len(insts):
                    inst = insts[i]
                    si = inst.sync_info
                    waits = list(si.on_wait) if (si and si.on_wait) else []
                    if len(waits) <= 1:
                        i += 1
                        continue
                    hoist = waits[1:]
                    for w in hoist:
                        if not (
                            w.sync_type == "semaphore"
                            and w.wait_mode == "sem-ge-imm"
                            and w.id in sem_by_num
                        ):
                            raise RuntimeError(
                                f"cannot hoist waits on {inst.name}: {waits}"
                            )
                    del si.on_wait[1:]
                    engine = nc.engines[inst.engine]
                    new_insts = []
                    for w in hoist:
                        wi = engine.wait_ge(sem_by_num[w.id], w.wait_value)
                        new_insts.append(wi.ins)
                    cur_list = nc.cur_bb.bb.instructions
                    for ni in new_insts:
                        cur_list.remove(ni)
                    insts[i:i] = new_insts
                    i += len(new_insts) + 1

    def _drain_and_barrier(self, tick_clock, wait_clock):
        nc = self.nc
        self._hoist_multi_waits()

        drain_inst = nc.sync.drain()
        wait_clock.add_sem_waits(
            drain_inst.ins, ScopedClock({None: tick_clock.global_clock})
        )
        waits = list(drain_inst.ins.sync_info.on_wait or [])
        if len(waits) > 1:
            drain_inst.ins.sync_info.on_wait.clear()
            assert self.sems is not None
            sem_by_num = {s.num: s for s in self.sems.allocated().values()}
            new_waits = []
            for w in waits:
                assert w.sync_type == "semaphore" and w.wait_mode == "sem-ge-imm", w
                new_waits.append(nc.sync.wait_ge(sem_by_num[w.id], w.wait_value))
            bb = nc.cur_bb.bb
            insts = bb.instructions
            names = [i.name for i in insts]
            di = names.index(drain_inst.ins.name)
            tail = insts[di + 1 : di + 1 + len(new_waits)]
            assert len(tail) == len(new_waits)
            insts[di : di + 1 + len(new_waits)] = tail + [drain_inst.ins]

        nc.all_engine_barrier()
        assert self.sems is not None
        popped = nc._tile_sem_poison_stack.pop()
        assert popped is self._sem_poison
        nc.clear_and_free_semaphores(list(self.sems.allocated().values()))
        nc.all_engine_barrier()


def ts(i, n=P):
    return slice(i * n, (i + 1) * n)


def build_nc():
    nc = bass.Bass()
    x_in = nc.dram_tensor("x", [BL, T, C], F32, kind="ExternalInput")
    wq_in = nc.dram_tensor("wqf", [C, C], BF16, kind="ExternalInput")
    wk_in = nc.dram_tensor("wkf", [C, C], BF16, kind="ExternalInput")
    wv_in = nc.dram_tensor("wvf", [C, C], BF16, kind="ExternalInput")
    wp_in = nc.dram_tensor("wpf", [C, C], BF16, kind="ExternalInput")
    w1_in = nc.dram_tensor("w1f", [C, F], BF16, kind="ExternalInput")
    w2_in = nc.dram_tensor("w2f", [F, C], BF16, kind="ExternalInput")
    gb_in = nc.dram_tensor("gb", [6, C], F32, kind="ExternalInput")
    b1_in = nc.dram_tensor("b1v", [F], F32, kind="ExternalInput")
    id_in = nc.dram_tensor("ident", [P, P], BF16, kind="ExternalInput")
    m_in = nc.dram_tensor("masks", [TT, P, T], BF16, kind="ExternalInput")
    out_t = nc.dram_tensor("out", [BL, T, C], F32, kind="ExternalOutput")

    IP = 2               # items per group
    NG = BL // IP        # groups
    W = IP * T           # moving width for feature-major matmuls (512)

    with PatchedTileContext(nc) as tc:
        with tc.tile_pool(name="consts", bufs=1) as consts:
            def load_w(ap_dram, kt, m, tag):
                w = consts.tile([P, kt, m], BF16, tag=tag)
                nc.sync.dma_start(w[:], ap_dram.rearrange("(kt p) m -> p kt m", p=P))
                return w

            wq_r = load_w(wq_in[:], CT, C, "wq")
            wk_r = load_w(wk_in[:], CT, C, "wk")
            wv_r = load_w(wv_in[:], CT, C, "wv")
            wp_r = load_w(wp_in[:], CT, C, "wp")
            w1_r = load_w(w1_in[:], CT, F, "w1")
            w2_r = load_w(w2_in[:], FT, C, "w2")

            id_b = consts.tile([P, P], BF16, tag="idb")
            nc.sync.dma_start(id_b[:], id_in[:])
            m_r = consts.tile([P, TT, T], BF16, tag="mr")
            nc.sync.dma_start(m_r[:], m_in.rearrange("tt p t -> p tt t"))
            gb = consts.tile([P, 6, CT], F32, tag="gb")
            nc.sync.dma_start(gb[:], gb_in.rearrange("g (ct p) -> p g ct", p=P))
            b1c = consts.tile([P, FT], F32, tag="b1c")
            nc.sync.dma_start(b1c[:], b1_in.rearrange("(ft p) -> p ft", p=P))
            epsc = consts.tile([P, 1], F32, tag="eps")
            nc.gpsimd.memset(epsc[:], LN_EPS)

            g1c = gb[:, 0, :]
            be1c = gb[:, 1, :]
            g2c = gb[:, 2, :]
            be2c = gb[:, 3, :]
            bpc = gb[:, 4, :]
            b2c = gb[:, 5, :]

            with (
                tc.tile_pool(name="xld", bufs=2) as xldp,
                tc.tile_pool(name="act", bufs=2) as actp,
                tc.tile_pool(name="xn", bufs=3) as xnp,
                tc.tile_pool(name="fm", bufs=2) as fmp,
                tc.tile_pool(name="zp", bufs=1) as zp,
                tc.tile_pool(name="attn", bufs=4) as attnp,
                tc.tile_pool(name="stats", bufs=8) as stats,
                tc.tile_pool(name="ps5", bufs=2, space="PSUM") as ps5,
                tc.tile_pool(name="ps2", bufs=2, space="PSUM") as ps2,
            ):
                def load_x(g):
                    xt = xldp.tile([P, IP, TT, C], F32, tag="x")
                    nc.sync.dma_start(
                        xt[:],
                        x_in[g * IP : (g + 1) * IP].rearrange(
                            "i (tt p) c -> p i tt c", p=P
                        ),
                    )
                    return xt

                def ln_stats_gen(src):
                    """src [P, IP, TT, C] f32 -> xn [P, 4, C] bf16 normalized
                    (no affine -- g/be folded into the ln_fm copies)."""
                    s4 = src.rearrange("p i tt c -> p (i tt) c")
                    nseg = IP * TT
                    bns = stats.tile([P, nseg, 6], F32, tag="bns")
                    for seg in range(nseg):
                        nc.vector.bn_stats(bns[:, seg, :], s4[:, seg, :])
                        if seg % 2 == 1:
                            yield
                    mv = stats.tile([P, nseg, 2], F32, tag="mv")
                    for seg in range(nseg):
                        nc.vector.bn_aggr(mv[:, seg, :], bns[:, seg, :])
                    yield
                    std = stats.tile([P, nseg], F32, tag="std")
                    nc.scalar.activation(std[:], mv[:, :, 1], AF.Sqrt, bias=epsc[:])
                    rstd = stats.tile([P, nseg], F32, tag="rstd")
                    nc.vector.reciprocal(rstd[:], std[:])
                    yield
                    xn = xnp.tile([P, nseg, C], BF16, tag="xn")
                    for seg in range(nseg):
                        nc.vector.tensor_scalar(
                            xn[:, seg, :], s4[:, seg, :],
                            mv[:, seg, 0:1],
                            rstd[:, seg : seg + 1],
                            ALU.subtract, ALU.mult,
                        )
                        if seg % 2 == 1:
                            yield
                    return xn

                def ln_fm(xn, gcol, becol):
                    """xn [P, 4, C] bf16 -> h_ct [P, CT, IP, T] bf16 w/ affine."""
                    h_ct = fmp.tile([P, CT, IP, T], BF16, tag="hct")
                    for ct in range(CT):
                        ps = ps5.tile([P, IP, T], BF16, tag="ps5t", bufs=2)
                        for i in range(IP):
                            for tt in range(TT):
                                nc.tensor.transpose(
                                    ps[:, i, ts(tt)],
                                    xn[:, i * TT + tt, ts(ct)],
                                    id_b[:],
                                )
                        nc.scalar.activation(
                            h_ct[:, ct, :, :], ps[:], AF.Identity,
                            bias=becol[:, ct : ct + 1],
                            scale=gcol[:, ct : ct + 1],
                        )
                        yield
                    return h_ct

                def front(g, x_t, xn1):
                    h_ct = yield from ln_fm(xn1, g1c, be1c)

                    qT = fmp.tile([P, CT, IP, T], BF16, tag="fmA")
                    kT = fmp.tile([P, CT, IP, T], BF16, tag="fmB")
                    for m in range(CT):
                        psq = ps5.tile([P, IP, T], F32, tag="ps5")
                        psk = ps5.tile([P, IP, T], F32, tag="ps5")
                        for k in range(CT):
                            nc.tensor.matmul(
                                psq[:], wq_r[:, k, ts(m)], h_ct[:, k, :, :],
                                start=(k == 0), stop=(k == CT - 1),
                            )
                            nc.tensor.matmul(
                                psk[:], wk_r[:, k, ts(m)], h_ct[:, k, :, :],
                                start=(k == 0), stop=(k == CT - 1),
                            )
                        nc.scalar.copy(qT[:, m, :, :], psq[:])
                        nc.vector.tensor_copy(kT[:, m, :, :], psk[:])
                        yield
                    v_sb = fmp.tile([P, IP, TT, C], BF16, tag="fmC")
                    for i in range(IP):
                        for st in range(TT):
                            psv = ps2.tile([P, C], F32, tag="ps2")
                            for k in range(CT):
                                nc.tensor.matmul(
                                    psv[:], h_ct[:, k, i, ts(st)], wv_r[:, k, :],
                                    start=(k == 0), stop=(k == CT - 1),
                                )
                            if st == 0:
                                nc.scalar.copy(v_sb[:, i, st, :], psv[:])
                            else:
                                nc.vector.tensor_copy(v_sb[:, i, st, :], psv[:])
                            yield

                    attnT = fmp.tile([P, CT, IP, T], BF16, tag="fmC2")
                    for i in range(IP):
                        for j in range(H2):
                            wtp = attnp.tile([P, 2, TT, T], BF16, tag="wtp")
                            for hh in range(2):
                                h = 2 * j + hh
                                off = hh * 64
                                rowsum = stats.tile([P, 2], F32, tag="rs")
                                we0 = attnp.tile([P, P], BF16, tag="we0")
                                we1 = attnp.tile([P, T], BF16, tag="we1")
                                # one PSUM tile: [:, :128] = tt0 rows (only
                                # cols [0,128) live); [:, 128:384] = tt1 rows
                                pss = ps2.tile([P, P + T], F32, tag="pss")
                                nc.tensor.matmul(
                                    pss[:, :P],
                                    qT[off : off + 64, j, i, ts(0)],
                                    kT[off : off + 64, j, i, :P],
                                    start=True, stop=False,
                                )
                                nc.tensor.matmul(
                                    pss[:, :P], id_b[:], m_r[:, 0, :P],
                                    start=False, stop=True,
                                )
                                nc.scalar.activation(
                                    we0[:], pss[:, :P], AF.Exp,
                                    scale=CSCALE, accum_out=rowsum[:, 0:1],
                                )
                                nc.tensor.matmul(
                                    pss[:, P : 2 * P],
                                    qT[off : off + 64, j, i, ts(1)],
                                    kT[off : off + 64, j, i, :P],
                                    start=True, stop=True,
                                )
                                nc.tensor.matmul(
                                    pss[:, 2 * P :],
                                    qT[off : off + 64, j, i, ts(1)],
                                    kT[off : off + 64, j, i, P:],
                                    start=True, stop=False,
                                )
                                nc.tensor.matmul(
                                    pss[:, 2 * P :], id_b[:], m_r[:, 1, P:],
                                    start=False, stop=True,
                                )
                                nc.scalar.activation(
                                    we1[:], pss[:, P:], AF.Exp,
                                    scale=CSCALE, accum_out=rowsum[:, 1:2],
                                )
                                rec = stats.tile([P, 2], F32, tag="rec")
                                nc.vector.reciprocal(rec[:], rowsum[:])
                                dg = attnp.tile([P, 2, P], BF16, tag="dg")
                                nc.gpsimd.tensor_scalar_mul(
                                    dg[:, 0, :], id_b[:], rec[:, 0:1]
                                )
                                nc.gpsimd.tensor_scalar_mul(
                                    dg[:, 1, :], id_b[:], rec[:, 1:2]
                                )
                                yield
                                # normalized transpose: psw[s,t] = we[t,s]/Z_t
                                psw = ps2.tile([P, T], F32, tag="ps2")
                                nc.tensor.matmul(
                                    psw[:, ts(0)], we0[:], dg[:, 0, :],
                                    start=True, stop=True,
                                )
                                nc.tensor.matmul(
                                    psw[:, ts(1)], we1[:, :P], dg[:, 1, :],
                                    start=True, stop=True,
                                )
                                nc.vector.tensor_copy(wtp[:, hh, 0, :], psw[:])
                                psw2 = ps2.tile([P, P], F32, tag="ps2")
                                nc.tensor.matmul(
                                    psw2[:], we1[:, P:], dg[:, 1, :],
                                    start=True, stop=True,
                                )
                                nc.vector.tensor_copy(wtp[:, hh, 1, ts(1)], psw2[:])
                                yield
                            psa = ps2.tile([P, T], F32, tag="pss")
                            for hh in range(2):
                                h = 2 * j + hh
                                nc.tensor.matmul(
                                    psa[ts(hh, 64), :],
                                    v_sb[:, i, 0, h * 64 : (h + 1) * 64],
                                    wtp[:, hh, 0, :],
                                    start=True, stop=False,
                                )
                                nc.tensor.matmul(
                                    psa[ts(hh, 64), ts(1)],
                                    v_sb[:, i, 1, h * 64 : (h + 1) * 64],
                                    wtp[:, hh, 1, ts(1)],
                                    start=False, stop=True,
                                )
                            nc.vector.tensor_copy(attnT[:, j, i, :], psa[:])
                            yield

                    saT = fmp.tile([P, CT, IP, T], BF16, tag="fmA2")
                    for m in range(CT):
                        psj = ps5.tile([P, IP, T], F32, tag="ps5")
                        for k in range(CT):
                            nc.tensor.matmul(
                                psj[:], wp_r[:, k, ts(m)], attnT[:, k, :, :],
                                start=(k == 0), stop=(k == CT - 1),
                            )
                        nc.scalar.activation(
                            saT[:, m, :, :], psj[:], AF.Identity,
                            bias=bpc[:, m : m + 1],
                        )
                        yield
                    x1 = actp.tile([P, IP, TT, C], F32, tag="x1")
                    for i in range(IP):
                        for tt in range(TT):
                            psr = ps5.tile([P, C], BF16, tag="ps5t", bufs=2)
                            for ct in range(CT):
                                nc.tensor.transpose(
                                    psr[:, ts(ct)], saT[:, ct, i, ts(tt)], id_b[:]
                                )
                            nc.vector.tensor_tensor(
                                x1[:, i, tt, :], psr[:], x_t[:, i, tt, :], ALU.add
                            )
                            yield
                    xn2 = yield from ln_stats_gen(x1[:])
                    return x1, xn2

                def back(g, x1, xn2):
                    h2_ct = yield from ln_fm(xn2, g2c, be2c)
                    z = zp.tile([P, FT, IP, T], BF16, tag="z")
                    for m in range(FT):
                        psz = ps5.tile([P, IP, T], F32, tag="ps5")
                        for k in range(CT):
                            nc.tensor.matmul(
                                psz[:], w1_r[:, k, ts(m)], h2_ct[:, k, :, :],
                                start=(k == 0), stop=(k == CT - 1),
                            )
                        nc.scalar.activation(
                            z[:, m, :, :], psz[:], AF.Relu,
                            bias=b1c[:, m : m + 1],
                        )
                        yield
                    yT = fmp.tile([P, CT, IP, T], BF16, tag="fmB2")
                    for m in range(CT):
                        psy = ps5.tile([P, IP, T], F32, tag="ps5")
                        for k in range(FT):
                            nc.tensor.matmul(
                                psy[:], w2_r[:, k, ts(m)], z[:, k, :, :],
                                start=(k == 0), stop=(k == FT - 1),
                            )
                        nc.scalar.activation(
                            yT[:, m, :, :], psy[:], AF.Identity,
                            bias=b2c[:, m : m + 1],
                        )
                        yield
                    for i in range(IP):
                        for tt in range(TT):
                            pso = ps5.tile([P, C], BF16, tag="ps5t", bufs=2)
                            for ct in range(CT):
                                nc.tensor.transpose(
                                    pso[:, ts(ct)], yT[:, ct, i, ts(tt)], id_b[:]
                                )
                            o = actp.tile([P, C], F32, tag="o")
                            nc.vector.tensor_tensor(
                                o[:], pso[:], x1[:, i, tt, :], ALU.add
                            )
                            nc.sync.dma_start(out_t[g * IP + i, ts(tt), :], o[:])
                            yield

                def run_all(gens):
                    """Round-robin the generators; return list of returns."""
                    rets = {}
                    live = {id(gn): gn for gn in gens}
                    order = [id(gn) for gn in gens]
                    while live:
                        for key in list(order):
                            gn = live.get(key)
                            if gn is None:
                                continue
                            try:
                                next(gn)
                            except StopIteration as e:
                                rets[key] = e.value
                                del live[key]
                    return [rets[id(gn)] for gn in gens]

                x_cur = load_x(0)
                (xn_cur,) = run_all([ln_stats_gen(x_cur[:])])
                pending = None
                for g in range(NG):
                    x_nxt = load_x(g + 1) if g + 1 < NG else None
                    gens = [front(g, x_cur, xn_cur)]
                    if x_nxt is not None:
                        gens.append(ln_stats_gen(x_nxt[:]))
                    if pending is not None:
                        gens.append(back(*pending))
                    rets = run_all(gens)
                    pending = (g,) + rets[0]
                    xn_cur = rets[1] if x_nxt is not None else None
                    x_cur = x_nxt
                run_all([back(*pending)])
    return nc


_NC_CACHE = None


def _get_nc():
    global _NC_CACHE
    if _NC_CACHE is None:
        _NC_CACHE = build_nc()
    return _NC_CACHE


def _host_consts():
    ident = np.eye(P, dtype=ml_dtypes.bfloat16)
    masks = np.zeros((TT, P, T), dtype=np.float32)
    for tt in range(TT):
        trow = np.arange(P) + tt * P
        scol = np.arange(T)
        masks[tt][scol[None, :] > trow[:, None]] = NEG
    return ident, masks.astype(ml_dtypes.bfloat16)


def kernel(x, Wq, Wk, Wv, Wp, bp, W1, b1, W2, b2, g1, be1, g2, be2):
    bf = ml_dtypes.bfloat16
    x = np.ascontiguousarray(np.asarray(x, np.float32))
    WqF = np.ascontiguousarray(
        np.asarray(Wq, np.float32).transpose(1, 0, 2).reshape(C, C).astype(bf)
    )
    WkF = np.ascontiguousarray(
        np.asarray(Wk, np.float32).transpose(1, 0, 2).reshape(C, C).astype(bf)
    )
    WvF = np.ascontiguousarray(
        np.asarray(Wv, np.float32).transpose(1, 0, 2).reshape(C, C).astype(bf)
    )
    WpF = np.ascontiguousarray(np.asarray(Wp, np.float32).astype(bf))
    W1F = np.ascontiguousarray(np.asarray(W1, np.float32).astype(bf))
    W2F = np.ascontiguousarray(np.asarray(W2, np.float32).astype(bf))
    gb = np.ascontiguousarray(
        np.stack([
            np.asarray(g1, np.float32), np.asarray(be1, np.float32),
            np.asarray(g2, np.float32), np.asarray(be2, np.float32),
            np.asarray(bp, np.float32), np.asarray(b2, np.float32),
        ])
    )
    b1v = np.ascontiguousarray(np.asarray(b1, np.float32))
    ident, masks = _host_consts()

    nc = _get_nc()
    shared = {
        "wqf": WqF, "wkf": WkF, "wvf": WvF, "wpf": WpF,
        "w1f": W1F, "w2f": W2F, "gb": gb, "b1v": b1v,
        "ident": ident, "masks": masks,
    }
    in_maps = []
    for c in range(NCORES):
        m = dict(shared)
        m["x"] = np.ascontiguousarray(x[c * BL : (c + 1) * BL])
        in_maps.append(m)

    from concourse.bass_utils import run_bass_kernel_spmd

    res = run_bass_kernel_spmd(nc, in_maps, list(range(NCORES)))
    out = np.concatenate([res.results[c]["out"] for c in range(NCORES)], axis=0)
    return out.astype(np.float32)


# revision 10
# speedup vs baseline: 1.6790x; 1.6790x over previous
"""Transformer block (dense_transformer) on 8 TRN2 NeuronCores.

Strategy: pure data-parallel over batch (B=128 -> 16 items/core), weights
replicated. Per item, all linear layers run feature-major ([feat, T] with
feat on partitions); LayerNorm/softmax run token-major ([T, feat]).
Matmul datapath is bf16 (2x PE stream rate vs f32r); residual stream and
softmax statistics stay f32. The softmax normalize is fused into the
weight transpose on PE via a diagonal 1/rowsum matrix built on GpSimd.
LN stats for group g+1 are computed during group g so PE never idles at
group boundaries.
"""

import numpy as np
import ml_dtypes

import concourse.bass as bass
import concourse.mybir as mybir
from concourse.tile import TileContext
from concourse.vector_clock import ScopedClock

F32 = mybir.dt.float32
BF16 = mybir.dt.bfloat16
AF = mybir.ActivationFunctionType
AX = mybir.AxisListType
ALU = mybir.AluOpType

B, T, C, H, D = 128, 256, 384, 6, 64
F = 4 * C
NCORES = 8
BL = B // NCORES
P = 128
TT = T // P    # 2 token tiles
CT = C // P    # 3 channel tiles
FT = F // P    # 12 ffn-hidden tiles
H2 = H // 2    # head pairs
LN_EPS = 1e-5
CSCALE = float(C) ** -0.5
NEG = -1.0e9


class PatchedTileContext(TileContext):
    """Workaround for this container's walrus: BIR instructions may carry at
    most ONE attached sem wait. Hoist extras into standalone waits."""

    def _hoist_multi_waits(self):
        nc = self.nc
        assert self.sems is not None
        sem_by_num = {s.num: s for s in self.sems.allocated().values()}
        for func in nc.m.functions:
            for blk in func.blocks:
                insts = blk.instructions
                i = 0
                while i < len(insts):
                    inst = insts[i]
                    si = inst.sync_info
                    waits = list(si.on_wait) if (si and si.on_wait) else []
                    if len(waits) <= 1:
                        i += 1
                        continue
                    hoist = waits[1:]
                    for w in hoist:
                        if not (
                            w.sync_type == "semaphore"
                            and w.wait_mode == "sem-ge-imm"
                            and w.id in sem_by_num
                        ):
                            raise RuntimeError(
                                f"cannot hoist waits on {inst.name}: {waits}"
                            )
                    del si.on_wait[1:]
                    engine = nc.engines[inst.engine]
                    new_insts = []
                    for w in hoist:
                        wi = engine.wait_ge(sem_by_num[w.id], w.wait_value)
                        new_insts.append(wi.ins)
                    cur_list = nc.cur_bb.bb.instructions
                    for ni in new_insts:
                        cur_list.remove(ni)
                    insts[i:i] = new_insts
                    i += len(new_insts) + 1

    def _drain_and_barrier(self, tick_clock, wait_clock):
        nc = self.nc
        self._hoist_multi_waits()

        drain_inst = nc.sync.drain()
        wait_clock.add_sem_waits(
            drain_inst.ins, ScopedClock({None: tick_clock.global_clock})
        )
        waits = list(drain_inst.ins.sync_info.on_wait or [])
        if len(waits) > 1:
            drain_inst.ins.sync_info.on_wait.clear()
            assert self.sems is not None
            sem_by_num = {s.num: s for s in self.sems.allocated().values()}
            new_waits = []
            for w in waits:
                assert w.sync_type == "semaphore" and w.wait_mode == "sem-ge-imm", w
                new_waits.append(nc.sync.wait_ge(sem_by_num[w.id], w.wait_value))
            bb = nc.cur_bb.bb
            insts = bb.instructions
            names = [i.name for i in insts]
            di = names.index(drain_inst.ins.name)
            tail = insts[di + 1 : di + 1 + len(new_waits)]
            assert len(tail) == len(new_waits)
            insts[di : di + 1 + len(new_waits)] = tail + [drain_inst.ins]

        nc.all_engine_barrier()
        assert self.sems is not None
        popped = nc._tile_sem_poison_stack.pop()
        assert popped is self._sem_poison
        nc.clear_and_free_semaphores(list(self.sems.allocated().values()))
        nc.all_engine_barrier()


def ts(i, n=P):
    return slice(i * n, (i + 1) * n)


def build_nc():
    nc = bass.Bass()
    x_in = nc.dram_tensor("x", [BL, T, C], F32, kind="ExternalInput")
    wq_in = nc.dram_tensor("wqf", [C, C], BF16, kind="ExternalInput")
    wk_in = nc.dram_tensor("wkf", [C, C], BF16, kind="ExternalInput")
    wv_in = nc.dram_tensor("wvf", [C, C], BF16, kind="ExternalInput")
    wp_in = nc.dram_tensor("wpf", [C, C], BF16, kind="ExternalInput")
    w1_in = nc.dram_tensor("w1f", [C, F], BF16, kind="ExternalInput")
    w2_in = nc.dram_tensor("w2f", [F, C], BF16, kind="ExternalInput")
    gb_in = nc.dram_tensor("gb", [6, C], F32, kind="ExternalInput")
    b1_in = nc.dram_tensor("b1v", [F], F32, kind="ExternalInput")
    id_in = nc.dram_tensor("ident", [P, P], BF16, kind="ExternalInput")
    m_in = nc.dram_tensor("masks", [TT, P, T], BF16, kind="ExternalInput")
    out_t = nc.dram_tensor("out", [BL, T, C], F32, kind="ExternalOutput")

    IP = 2               # items per group
    NG = BL // IP        # groups
    W = IP * T           # moving width for feature-major matmuls (512)

    with PatchedTileContext(nc) as tc:
        with tc.tile_pool(name="consts", bufs=1) as consts:
            def load_w(ap_dram, kt, m, tag):
                w = consts.tile([P, kt, m], BF16, tag=tag)
                nc.sync.dma_start(w[:], ap_dram.rearrange("(kt p) m -> p kt m", p=P))
                return w

            wq_r = load_w(wq_in[:], CT, C, "wq")
            wk_r = load_w(wk_in[:], CT, C, "wk")
            wv_r = load_w(wv_in[:], CT, C, "wv")
            wp_r = load_w(wp_in[:], CT, C, "wp")
            w1_r = load_w(w1_in[:], CT, F, "w1")
            w2_r = load_w(w2_in[:], FT, C, "w2")

            id_b = consts.tile([P, P], BF16, tag="idb")
            nc.sync.dma_start(id_b[:], id_in[:])
            m_r = consts.tile([P, TT, T], BF16, tag="mr")
            nc.sync.dma_start(m_r[:], m_in.rearrange("tt p t -> p tt t"))
            gb = consts.tile([P, 6, CT], F32, tag="gb")
            nc.sync.dma_start(gb[:], gb_in.rearrange("g (ct p) -> p g ct", p=P))
            b1c = consts.tile([P, FT], F32, tag="b1c")
            nc.sync.dma_start(b1c[:], b1_in.rearrange("(ft p) -> p ft", p=P))
            epsc = consts.tile([P, 1], F32, tag="eps")
            nc.gpsimd.memset(epsc[:], LN_EPS)

            g1c = gb[:, 0, :]
            be1c = gb[:, 1, :]
            g2c = gb[:, 2, :]
            be2c = gb[:, 3, :]
            bpc = gb[:, 4, :]
            b2c = gb[:, 5, :]

            with (
                tc.tile_pool(name="xld", bufs=2) as xldp,
                tc.tile_pool(name="act", bufs=2) as actp,
                tc.tile_pool(name="xn", bufs=3) as xnp,
                tc.tile_pool(name="fm", bufs=2) as fmp,
                tc.tile_pool(name="zp", bufs=1) as zp,
                tc.tile_pool(name="attn", bufs=4) as attnp,
                tc.tile_pool(name="stats", bufs=8) as stats,
                tc.tile_pool(name="ps5", bufs=2, space="PSUM") as ps5,
                tc.tile_pool(name="ps2", bufs=2, space="PSUM") as ps2,
            ):
                def load_x(g):
                    xt = xldp.tile([P, IP, TT, C], F32, tag="x")
                    nc.sync.dma_start(
                        xt[:],
                        x_in[g * IP : (g + 1) * IP].rearrange(
                            "i (tt p) c -> p i tt c", p=P
                        ),
                    )
                    return xt

                def ln_stats_gen(src):
                    """src [P, IP, TT, C] f32 -> xn [P, 4, C] bf16 normalized
                    (no affine -- g/be folded into the ln_fm copies)."""
                    s4 = src.rearrange("p i tt c -> p (i tt) c")
                    nseg = IP * TT
                    bns = stats.tile([P, nseg, 6], F32, tag="bns")
                    for seg in range(nseg):
                        nc.vector.bn_stats(bns[:, seg, :], s4[:, seg, :])
                        if seg % 2 == 1:
                            yield
                    mv = stats.tile([P, nseg, 2], F32, tag="mv")
                    for seg in range(nseg):
                        nc.vector.bn_aggr(mv[:, seg, :], bns[:, seg, :])
                    yield
                    lnv = stats.tile([P, nseg], F32, tag="lnv")
                    nc.scalar.activation(lnv[:], mv[:, :, 1], AF.Ln, bias=epsc[:])
                    rstd = stats.tile([P, nseg], F32, tag="rstd")
                    nc.scalar.activation(rstd[:], lnv[:], AF.Exp, scale=-0.5)
                    yield
                    xn = xnp.tile([P, nseg, C], BF16, tag="xn")
                    for seg in range(nseg):
                        nc.vector.tensor_scalar(
                            xn[:, seg, :], s4[:, seg, :],
                            mv[:, seg, 0:1],
                            rstd[:, seg : seg + 1],
                            ALU.subtract, ALU.mult,
                        )
                        if seg % 2 == 1:
                            yield
                    return xn

                def ln_fm(xn, gcol, becol):
                    """xn [P, 4, C] bf16 -> h_ct [P, CT, IP, T] bf16 w/ affine."""
                    h_ct = fmp.tile([P, CT, IP, T], BF16, tag="hct")
                    for ct in range(CT):
                        ps = ps5.tile([P, IP, T], BF16, tag="ps5t", bufs=2)
                        for i in range(IP):
                            for tt in range(TT):
                                nc.tensor.transpose(
                                    ps[:, i, ts(tt)],
                                    xn[:, i * TT + tt, ts(ct)],
                                    id_b[:],
                                )
                        nc.scalar.activation(
                            h_ct[:, ct, :, :], ps[:], AF.Identity,
                            bias=becol[:, ct : ct + 1],
                            scale=gcol[:, ct : ct + 1],
                        )
                        yield
                    return h_ct

                def front(g, x_t, xn1):
                    h_ct = yield from ln_fm(xn1, g1c, be1c)

                    qT = fmp.tile([P, CT, IP, T], BF16, tag="fmA")
                    kT = fmp.tile([P, CT, IP, T], BF16, tag="fmB")
                    for m in range(CT):
                        psq = ps5.tile([P, IP, T], F32, tag="ps5")
                        psk = ps5.tile([P, IP, T], F32, tag="ps5")
                        for k in range(CT):
                            nc.tensor.matmul(
                                psq[:], wq_r[:, k, ts(m)], h_ct[:, k, :, :],
                                start=(k == 0), stop=(k == CT - 1),
                            )
                            nc.tensor.matmul(
                                psk[:], wk_r[:, k, ts(m)], h_ct[:, k, :, :],
                                start=(k == 0), stop=(k == CT - 1),
                            )
                        nc.scalar.copy(qT[:, m, :, :], psq[:])
                        nc.vector.tensor_copy(kT[:, m, :, :], psk[:])
                        yield
                    v_sb = fmp.tile([P, IP, TT, C], BF16, tag="fmC")
                    for i in range(IP):
                        for st in range(TT):
                            psv = ps2.tile([P, C], F32, tag="ps2")
                            for k in range(CT):
                                nc.tensor.matmul(
                                    psv[:], h_ct[:, k, i, ts(st)], wv_r[:, k, :],
                                    start=(k == 0), stop=(k == CT - 1),
                                )
                            if st == 0:
                                nc.scalar.copy(v_sb[:, i, st, :], psv[:])
                            else:
                                nc.vector.tensor_copy(v_sb[:, i, st, :], psv[:])
                            yield

                    attnT = fmp.tile([P, CT, IP, T], BF16, tag="fmC2")
                    for i in range(IP):
                        for j in range(H2):
                            # wtp[:, hh, 0:2, :] = [s 0:128, t 0:256];
                            # wtp[:, hh, 2, :]   = [s 128:256, t 128:256]
                            wtp = attnp.tile([P, 2, 3, P], BF16, tag="wtp")
                            for hh in range(2):
                                h = 2 * j + hh
                                off = hh * 64
                                rowsum = stats.tile([P, 2], F32, tag="rs")
                                we0 = attnp.tile([P, P], BF16, tag="we0")
                                we1 = attnp.tile([P, T], BF16, tag="we1")
                                # one PSUM tile: [:, :128] = tt0 rows (only
                                # cols [0,128) live); [:, 128:384] = tt1 rows
                                pss = ps2.tile([P, P + T], F32, tag="pss")
                                nc.tensor.matmul(
                                    pss[:, :P],
                                    qT[off : off + 64, j, i, ts(0)],
                                    kT[off : off + 64, j, i, :P],
                                    start=True, stop=False,
                                )
                                nc.tensor.matmul(
                                    pss[:, :P], id_b[:], m_r[:, 0, :P],
                                    start=False, stop=True,
                                )
                                nc.scalar.activation(
                                    we0[:], pss[:, :P], AF.Exp,
                                    scale=CSCALE, accum_out=rowsum[:, 0:1],
                                )
                                nc.tensor.matmul(
                                    pss[:, P : 2 * P],
                                    qT[off : off + 64, j, i, ts(1)],
                                    kT[off : off + 64, j, i, :P],
                                    start=True, stop=True,
                                )
                                nc.tensor.matmul(
                                    pss[:, 2 * P :],
                                    qT[off : off + 64, j, i, ts(1)],
                                    kT[off : off + 64, j, i, P:],
                                    start=True, stop=False,
                                )
                                nc.tensor.matmul(
                                    pss[:, 2 * P :], id_b[:], m_r[:, 1, P:],
                                    start=False, stop=True,
                                )
                                nc.scalar.activation(
                                    we1[:], pss[:, P:], AF.Exp,
                                    scale=CSCALE, accum_out=rowsum[:, 1:2],
                                )
                                rec = stats.tile([P, 2], F32, tag="rec")
                                nc.vector.reciprocal(rec[:], rowsum[:])
                                dg = attnp.tile([P, 2, P], BF16, tag="dg")
                                nc.vector.tensor_scalar_mul(
                                    dg[:, 0, :], id_b[:], rec[:, 0:1]
                                )
                                nc.vector.tensor_scalar_mul(
                                    dg[:, 1, :], id_b[:], rec[:, 1:2]
                                )
                                yield
                                # normalized transpose: psw[s,t] = we[t,s]/Z_t
                                psw = ps2.tile([P, 3 * P], F32, tag="ps2")
                                nc.tensor.matmul(
                                    psw[:, ts(0)], we0[:], dg[:, 0, :],
                                    start=True, stop=True,
                                )
                                nc.tensor.matmul(
                                    psw[:, ts(1)], we1[:, :P], dg[:, 1, :],
                                    start=True, stop=True,
                                )
                                nc.tensor.matmul(
                                    psw[:, ts(2)], we1[:, P:], dg[:, 1, :],
                                    start=True, stop=True,
                                )
                                nc.vector.tensor_copy(
                                    wtp[:, hh, :, :],
                                    psw.rearrange("p (b f) -> p b f", b=3),
                                )
                                yield
                            psa = ps2.tile([P, T], F32, tag="pss")
                            for hh in range(2):
                                h = 2 * j + hh
                                nc.tensor.matmul(
                                    psa[ts(hh, 64), :],
                                    v_sb[:, i, 0, h * 64 : (h + 1) * 64],
                                    wtp[:, hh, 0:2, :].rearrange(
                                        "p b f -> p (b f)"),
                                    start=True, stop=False,
                                )
                                nc.tensor.matmul(
                                    psa[ts(hh, 64), ts(1)],
                                    v_sb[:, i, 1, h * 64 : (h + 1) * 64],
                                    wtp[:, hh, 2, :],
                                    start=False, stop=True,
                                )
                            nc.vector.tensor_copy(attnT[:, j, i, :], psa[:])
                            yield

                    saT = fmp.tile([P, CT, IP, T], BF16, tag="fmA2")
                    for m in range(CT):
                        psj = ps5.tile([P, IP, T], F32, tag="ps5")
                        for k in range(CT):
                            nc.tensor.matmul(
                                psj[:], wp_r[:, k, ts(m)], attnT[:, k, :, :],
                                start=(k == 0), stop=(k == CT - 1),
                            )
                        nc.scalar.activation(
                            saT[:, m, :, :], psj[:], AF.Identity,
                            bias=bpc[:, m : m + 1],
                        )
                        yield
                    x1 = actp.tile([P, IP, TT, C], F32, tag="x1")
                    for i in range(IP):
                        for tt in range(TT):
                            psr = ps5.tile([P, C], BF16, tag="ps5t", bufs=2)
                            for ct in range(CT):
                                nc.tensor.transpose(
                                    psr[:, ts(ct)], saT[:, ct, i, ts(tt)], id_b[:]
                                )
                            nc.vector.tensor_tensor(
                                x1[:, i, tt, :], psr[:], x_t[:, i, tt, :], ALU.add
                            )
                            yield
                    xn2 = yield from ln_stats_gen(x1[:])
                    return x1, xn2

                def back(g, x1, xn2):
                    h2_ct = yield from ln_fm(xn2, g2c, be2c)
                    z = zp.tile([P, FT, IP, T], BF16, tag="z")
                    for m in range(FT):
                        psz = ps5.tile([P, IP, T], F32, tag="ps5")
                        for k in range(CT):
                            nc.tensor.matmul(
                                psz[:], w1_r[:, k, ts(m)], h2_ct[:, k, :, :],
                                start=(k == 0), stop=(k == CT - 1),
                            )
                        if m % 2 == 0:
                            nc.scalar.activation(
                                z[:, m, :, :], psz[:], AF.Relu,
                                bias=b1c[:, m : m + 1],
                            )
                        else:
                            nc.vector.tensor_scalar(
                                z[:, m, :, :], psz[:],
                                b1c[:, m : m + 1], 0.0,
                                ALU.add, ALU.max,
                            )
                        yield
                    yT = fmp.tile([P, CT, IP, T], BF16, tag="fmB2")
                    for m in range(CT):
                        psy = ps5.tile([P, IP, T], F32, tag="ps5")
                        for k in range(FT):
                            nc.tensor.matmul(
                                psy[:], w2_r[:, k, ts(m)], z[:, k, :, :],
                                start=(k == 0), stop=(k == FT - 1),
                            )
                        nc.scalar.activation(
                            yT[:, m, :, :], psy[:], AF.Identity,
                            bias=b2c[:, m : m + 1],
                        )
                        yield
                    for i in range(IP):
                        for tt in range(TT):
                            pso = ps5.tile([P, C], BF16, tag="ps5t", bufs=2)
                            for ct in range(CT):
                                nc.tensor.transpose(
                                    pso[:, ts(ct)], yT[:, ct, i, ts(tt)], id_b[:]
                                )
                            o = actp.tile([P, C], F32, tag="o")
                            nc.vector.tensor_tensor(
                                o[:], pso[:], x1[:, i, tt, :], ALU.add
                            )
                            nc.sync.dma_start(out_t[g * IP + i, ts(tt), :], o[:])
                            yield

                def run_all(gens):
                    """Round-robin the generators; return list of returns."""
                    rets = {}
                    live = {id(gn): gn for gn in gens}
                    order = [id(gn) for gn in gens]
                    while live:
                        for key in list(order):
                            gn = live.get(key)
                            if gn is None:
                                continue
                            try:
                                next(gn)
                            except StopIteration as e:
                                rets[key] = e.value
                                del live[key]
                    return [rets[id(gn)] for gn in gens]

                x_cur = load_x(0)
                (xn_cur,) = run_all([ln_stats_gen(x_cur[:])])
                pending = None
                for g in range(NG):
                    x_nxt = load_x(g + 1) if g + 1 < NG else None
                    gens = [front(g, x_cur, xn_cur)]
                    if x_nxt is not None:
                        gens.append(ln_stats_gen(x_nxt[:]))
                    if pending is not None:
                        gens.append(back(*pending))
                    rets = run_all(gens)
                    pending = (g,) + rets[0]
                    xn_cur = rets[1] if x_nxt is not None else None
                    x_cur = x_nxt
                run_all([back(*pending)])
    return nc


_NC_CACHE = None


def _get_nc():
    global _NC_CACHE
    if _NC_CACHE is None:
        _NC_CACHE = build_nc()
    return _NC_CACHE


def _host_consts():
    ident = np.eye(P, dtype=ml_dtypes.bfloat16)
    masks = np.zeros((TT, P, T), dtype=np.float32)
    for tt in range(TT):
        trow = np.arange(P) + tt * P
        scol = np.arange(T)
        masks[tt][scol[None, :] > trow[:, None]] = NEG
    return ident, masks.astype(ml_dtypes.bfloat16)


def kernel(x, Wq, Wk, Wv, Wp, bp, W1, b1, W2, b2, g1, be1, g2, be2):
    bf = ml_dtypes.bfloat16
    x = np.ascontiguousarray(np.asarray(x, np.float32))
    WqF = np.ascontiguousarray(
        np.asarray(Wq, np.float32).transpose(1, 0, 2).reshape(C, C).astype(bf)
    )
    WkF = np.ascontiguousarray(
        np.asarray(Wk, np.float32).transpose(1, 0, 2).reshape(C, C).astype(bf)
    )
    WvF = np.ascontiguousarray(
        np.asarray(Wv, np.float32).transpose(1, 0, 2).reshape(C, C).astype(bf)
    )
    WpF = np.ascontiguousarray(np.asarray(Wp, np.float32).astype(bf))
    W1F = np.ascontiguousarray(np.asarray(W1, np.float32).astype(bf))
    W2F = np.ascontiguousarray(np.asarray(W2, np.float32).astype(bf))
    gb = np.ascontiguousarray(
        np.stack([
            np.asarray(g1, np.float32), np.asarray(be1, np.float32),
            np.asarray(g2, np.float32), np.asarray(be2, np.float32),
            np.asarray(bp, np.float32), np.asarray(b2, np.float32),
        ])
    )
    b1v = np.ascontiguousarray(np.asarray(b1, np.float32))
    ident, masks = _host_consts()

    nc = _get_nc()
    shared = {
        "wqf": WqF, "wkf": WkF, "wvf": WvF, "wpf": WpF,
        "w1f": W1F, "w2f": W2F, "gb": gb, "b1v": b1v,
        "ident": ident, "masks": masks,
    }
    in_maps = []
    for c in range(NCORES):
        m = dict(shared)
        m["x"] = np.ascontiguousarray(x[c * BL : (c + 1) * BL])
        in_maps.append(m)

    from concourse.bass_utils import run_bass_kernel_spmd

    res = run_bass_kernel_spmd(nc, in_maps, list(range(NCORES)))
    out = np.concatenate([res.results[c]["out"] for c in range(NCORES)], axis=0)
    return out.astype(np.float32)


# revision 12
# speedup vs baseline: 1.9110x; 1.1382x over previous
"""Transformer block (dense_transformer) on 8 TRN2 NeuronCores.

Strategy: pure data-parallel over batch (B=128 -> 16 items/core), weights
replicated. Per item, all linear layers run feature-major ([feat, T] with
feat on partitions); LayerNorm/softmax run token-major ([T, feat]).
Matmul datapath is bf16 (2x PE stream rate vs f32r); residual stream and
softmax statistics stay f32. The softmax normalize is fused into the
weight transpose on PE via a diagonal 1/rowsum matrix built on GpSimd.
LN stats for group g+1 are computed during group g so PE never idles at
group boundaries.
"""

import numpy as np
import ml_dtypes

import concourse.bass as bass
import concourse.mybir as mybir
from concourse.tile import TileContext
from concourse.vector_clock import ScopedClock

F32 = mybir.dt.float32
BF16 = mybir.dt.bfloat16
AF = mybir.ActivationFunctionType
AX = mybir.AxisListType
ALU = mybir.AluOpType

B, T, C, H, D = 128, 256, 384, 6, 64
F = 4 * C
NCORES = 8
BL = B // NCORES
P = 128
TT = T // P    # 2 token tiles
CT = C // P    # 3 channel tiles
FT = F // P    # 12 ffn-hidden tiles
H2 = H // 2    # head pairs
LN_EPS = 1e-5
CSCALE = float(C) ** -0.5
NEG = -1.0e9


class PatchedTileContext(TileContext):
    """Workaround for this container's walrus: BIR instructions may carry at
    most ONE attached sem wait. Hoist extras into standalone waits."""

    def _hoist_multi_waits(self):
        nc = self.nc
        assert self.sems is not None
        sem_by_num = {s.num: s for s in self.sems.allocated().values()}
        for func in nc.m.functions:
            for blk in func.blocks:
                insts = blk.instructions
                i = 0
                while i < len(insts):
                    inst = insts[i]
                    si = inst.sync_info
                    waits = list(si.on_wait) if (si and si.on_wait) else []
                    if len(waits) <= 1:
                        i += 1
                        continue
                    hoist = waits[1:]
                    for w in hoist:
                        if not (
                            w.sync_type == "semaphore"
                            and w.wait_mode == "sem-ge-imm"
                            and w.id in sem_by_num
                        ):
                            raise RuntimeError(
                                f"cannot hoist waits on {inst.name}: {waits}"
                            )
                    del si.on_wait[1:]
                    engine = nc.engines[inst.engine]
                    new_insts = []
                    for w in hoist:
                        wi = engine.wait_ge(sem_by_num[w.id], w.wait_value)
                        new_insts.append(wi.ins)
                    cur_list = nc.cur_bb.bb.instructions
                    for ni in new_insts:
                        cur_list.remove(ni)
                    insts[i:i] = new_insts
                    i += len(new_insts) + 1

    def _drain_and_barrier(self, tick_clock, wait_clock):
        nc = self.nc
        self._hoist_multi_waits()

        drain_inst = nc.sync.drain()
        wait_clock.add_sem_waits(
            drain_inst.ins, ScopedClock({None: tick_clock.global_clock})
        )
        waits = list(drain_inst.ins.sync_info.on_wait or [])
        if len(waits) > 1:
            drain_inst.ins.sync_info.on_wait.clear()
            assert self.sems is not None
            sem_by_num = {s.num: s for s in self.sems.allocated().values()}
            new_waits = []
            for w in waits:
                assert w.sync_type == "semaphore" and w.wait_mode == "sem-ge-imm", w
                new_waits.append(nc.sync.wait_ge(sem_by_num[w.id], w.wait_value))
            bb = nc.cur_bb.bb
            insts = bb.instructions
            names = [i.name for i in insts]
            di = names.index(drain_inst.ins.name)
            tail = insts[di + 1 : di + 1 + len(new_waits)]
            assert len(tail) == len(new_waits)
            insts[di : di + 1 + len(new_waits)] = tail + [drain_inst.ins]

        nc.all_engine_barrier()
        assert self.sems is not None
        popped = nc._tile_sem_poison_stack.pop()
        assert popped is self._sem_poison
        nc.clear_and_free_semaphores(list(self.sems.allocated().values()))
        nc.all_engine_barrier()


def ts(i, n=P):
    return slice(i * n, (i + 1) * n)


def build_nc():
    nc = bass.Bass()
    x_in = nc.dram_tensor("x", [BL, T, C], F32, kind="ExternalInput")
    wq_in = nc.dram_tensor("wqf", [C, C], BF16, kind="ExternalInput")
    wk_in = nc.dram_tensor("wkf", [C, C], BF16, kind="ExternalInput")
    wv_in = nc.dram_tensor("wvf", [C, C], BF16, kind="ExternalInput")
    wp_in = nc.dram_tensor("wpf", [C, C], BF16, kind="ExternalInput")
    w1_in = nc.dram_tensor("w1f", [C, F], BF16, kind="ExternalInput")
    w2_in = nc.dram_tensor("w2f", [F, C], BF16, kind="ExternalInput")
    gb_in = nc.dram_tensor("gb", [6, C], F32, kind="ExternalInput")
    b1_in = nc.dram_tensor("b1v", [F], F32, kind="ExternalInput")
    id_in = nc.dram_tensor("ident", [P, P], BF16, kind="ExternalInput")
    m_in = nc.dram_tensor("masks", [TT, P, T], BF16, kind="ExternalInput")
    out_t = nc.dram_tensor("out", [BL, T, C], F32, kind="ExternalOutput")

    IP = 2               # items per group
    NG = BL // IP        # groups
    W = IP * T           # moving width for feature-major matmuls (512)

    with PatchedTileContext(nc) as tc:
        with tc.tile_pool(name="consts", bufs=1) as consts:
            def load_w(ap_dram, kt, m, tag):
                w = consts.tile([P, kt, m], BF16, tag=tag)
                nc.sync.dma_start(w[:], ap_dram.rearrange("(kt p) m -> p kt m", p=P))
                return w

            wq_r = load_w(wq_in[:], CT, C, "wq")
            wk_r = load_w(wk_in[:], CT, C, "wk")
            wv_r = load_w(wv_in[:], CT, C, "wv")
            wp_r = load_w(wp_in[:], CT, C, "wp")
            w1_r = load_w(w1_in[:], CT, F, "w1")
            w2_r = load_w(w2_in[:], FT, C, "w2")

            id_b = consts.tile([P, P], BF16, tag="idb")
            nc.sync.dma_start(id_b[:], id_in[:])
            m_r = consts.tile([P, TT, P], BF16, tag="mr")
            nc.sync.dma_start(m_r[:, 0, :], m_in[0, :, :P])
            nc.sync.dma_start(m_r[:, 1, :], m_in[1, :, P:])
            gb = consts.tile([P, 6, CT], F32, tag="gb")
            nc.sync.dma_start(gb[:], gb_in.rearrange("g (ct p) -> p g ct", p=P))
            b1c = consts.tile([P, FT], F32, tag="b1c")
            nc.sync.dma_start(b1c[:], b1_in.rearrange("(ft p) -> p ft", p=P))
            epsc = consts.tile([P, 1], F32, tag="eps")
            nc.gpsimd.memset(epsc[:], LN_EPS)

            g1c = gb[:, 0, :]
            be1c = gb[:, 1, :]
            g2c = gb[:, 2, :]
            be2c = gb[:, 3, :]
            bpc = gb[:, 4, :]
            b2c = gb[:, 5, :]

            with (
                tc.tile_pool(name="xld", bufs=3) as xldp,
                tc.tile_pool(name="act", bufs=2) as actp,
                tc.tile_pool(name="xn", bufs=3) as xnp,
                tc.tile_pool(name="fm", bufs=2) as fmp,
                tc.tile_pool(name="zp", bufs=1) as zp,
                tc.tile_pool(name="attn", bufs=4) as attnp,
                tc.tile_pool(name="stats", bufs=8) as stats,
                tc.tile_pool(name="ps5", bufs=2, space="PSUM") as ps5,
                tc.tile_pool(name="ps2", bufs=2, space="PSUM") as ps2,
            ):
                def load_x(g):
                    xt = xldp.tile([P, IP, TT, C], F32, tag="x")
                    nc.sync.dma_start(
                        xt[:],
                        x_in[g * IP : (g + 1) * IP].rearrange(
                            "i (tt p) c -> p i tt c", p=P
                        ),
                    )
                    return xt

                def ln_stats_gen(src):
                    """src [P, IP, TT, C] f32 -> xn [P, 4, C] bf16 normalized
                    (no affine -- g/be folded into the ln_fm copies)."""
                    s4 = src.rearrange("p i tt c -> p (i tt) c")
                    nseg = IP * TT
                    bns = stats.tile([P, nseg, 6], F32, tag="bns")
                    for seg in range(nseg):
                        nc.vector.bn_stats(bns[:, seg, :], s4[:, seg, :])
                        if seg % 2 == 1:
                            yield
                    mv = stats.tile([P, nseg, 2], F32, tag="mv")
                    for seg in range(nseg):
                        nc.vector.bn_aggr(mv[:, seg, :], bns[:, seg, :])
                    yield
                    lnv = stats.tile([P, nseg], F32, tag="lnv")
                    nc.scalar.activation(lnv[:], mv[:, :, 1], AF.Ln, bias=epsc[:])
                    rstd = stats.tile([P, nseg], F32, tag="rstd")
                    nc.scalar.activation(rstd[:], lnv[:], AF.Exp, scale=-0.5)
                    yield
                    xn = xnp.tile([P, nseg, C], BF16, tag="xn")
                    for seg in range(nseg):
                        nc.vector.tensor_scalar(
                            xn[:, seg, :], s4[:, seg, :],
                            mv[:, seg, 0:1],
                            rstd[:, seg : seg + 1],
                            ALU.subtract, ALU.mult,
                        )
                        if seg % 2 == 1:
                            yield
                    return xn

                def ln_fm(xn, gcol, becol):
                    """xn [P, 4, C] bf16 -> h_ct [P, CT, IP, T] bf16 w/ affine."""
                    h_ct = fmp.tile([P, CT, IP, T], BF16, tag="hct")
                    for ct in range(CT):
                        ps = ps5.tile([P, IP, T], BF16, tag="ps5t", bufs=2)
                        for i in range(IP):
                            for tt in range(TT):
                                nc.tensor.transpose(
                                    ps[:, i, ts(tt)],
                                    xn[:, i * TT + tt, ts(ct)],
                                    id_b[:],
                                )
                        nc.scalar.activation(
                            h_ct[:, ct, :, :], ps[:], AF.Identity,
                            bias=becol[:, ct : ct + 1],
                            scale=gcol[:, ct : ct + 1],
                        )
                        yield
                    return h_ct

                def front(g, x_t, xn1):
                    h_ct = yield from ln_fm(xn1, g1c, be1c)

                    qT = fmp.tile([P, CT, IP, T], BF16, tag="fmA")
                    kT = fmp.tile([P, CT, IP, T], BF16, tag="fmB")
                    for m in range(CT):
                        psq = ps5.tile([P, IP, T], F32, tag="ps5")
                        psk = ps5.tile([P, IP, T], F32, tag="ps5")
                        for k in range(CT):
                            nc.tensor.matmul(
                                psq[:], wq_r[:, k, ts(m)], h_ct[:, k, :, :],
                                start=(k == 0), stop=(k == CT - 1),
                            )
                            nc.tensor.matmul(
                                psk[:], wk_r[:, k, ts(m)], h_ct[:, k, :, :],
                                start=(k == 0), stop=(k == CT - 1),
                            )
                        nc.scalar.copy(qT[:, m, :, :], psq[:])
                        nc.vector.tensor_copy(kT[:, m, :, :], psk[:])
                        yield
                    v_sb = fmp.tile([P, IP, TT, C], BF16, tag="fmC")
                    for i in range(IP):
                        for st in range(TT):
                            psv = ps2.tile([P, C], F32, tag="ps2")
                            for k in range(CT):
                                nc.tensor.matmul(
                                    psv[:], h_ct[:, k, i, ts(st)], wv_r[:, k, :],
                                    start=(k == 0), stop=(k == CT - 1),
                                )
                            if st == 0:
                                nc.scalar.copy(v_sb[:, i, st, :], psv[:])
                            else:
                                nc.vector.tensor_copy(v_sb[:, i, st, :], psv[:])
                            yield

                    attnT = fmp.tile([P, CT, IP, T], BF16, tag="fmC2")
                    for i in range(IP):
                        for j in range(H2):
                            # wtp[:, hh, 0:2, :] = [s 0:128, t 0:256];
                            # wtp[:, hh, 2, :]   = [s 128:256, t 128:256]
                            wtp = attnp.tile([P, 2, 3, P], BF16, tag="wtp")
                            for hh in range(2):
                                h = 2 * j + hh
                                off = hh * 64
                                rowsum = stats.tile([P, 2], F32, tag="rs")
                                we0 = attnp.tile([P, P], BF16, tag="we0")
                                we1 = attnp.tile([P, T], BF16, tag="we1")
                                # pss layout: [:,:P]=tt0 diag block,
                                # [:,P:2P]=tt1 diag block (s in [128,256)),
                                # [:,2P:]=tt1 left block (s in [0,128)).
                                # One mask MM covers both diag blocks.
                                pss = ps2.tile([P, P + T], F32, tag="pss")
                                nc.tensor.matmul(
                                    pss[:, :P],
                                    qT[off : off + 64, j, i, ts(0)],
                                    kT[off : off + 64, j, i, :P],
                                    start=True, stop=False,
                                    skip_group_check=True,
                                )
                                nc.tensor.matmul(
                                    pss[:, P : 2 * P],
                                    qT[off : off + 64, j, i, ts(1)],
                                    kT[off : off + 64, j, i, P:],
                                    start=True, stop=False,
                                    skip_group_check=True,
                                )
                                nc.tensor.matmul(
                                    pss[:, : 2 * P],
                                    id_b[:],
                                    m_r.rearrange("p tt t -> p (tt t)"),
                                    start=False, stop=True,
                                    skip_group_check=True,
                                )
                                nc.scalar.activation(
                                    we0[:], pss[:, :P], AF.Exp,
                                    scale=CSCALE, accum_out=rowsum[:, 0:1],
                                )
                                nc.tensor.matmul(
                                    pss[:, 2 * P :],
                                    qT[off : off + 64, j, i, ts(1)],
                                    kT[off : off + 64, j, i, :P],
                                    start=True, stop=True,
                                )
                                nc.scalar.activation(
                                    we1[:], pss[:, P:], AF.Exp,
                                    scale=CSCALE, accum_out=rowsum[:, 1:2],
                                )
                                rec = stats.tile([P, 2], F32, tag="rec")
                                nc.vector.reciprocal(rec[:], rowsum[:])
                                dg = attnp.tile([P, 2, P], BF16, tag="dg")
                                nc.vector.tensor_scalar_mul(
                                    dg[:, 0, :], id_b[:], rec[:, 0:1]
                                )
                                nc.vector.tensor_scalar_mul(
                                    dg[:, 1, :], id_b[:], rec[:, 1:2]
                                )
                                yield
                                # normalized transpose: psw[s,t] = we[t,s]/Z_t
                                psw = ps2.tile([P, 3 * P], F32, tag="ps2")
                                nc.tensor.matmul(
                                    psw[:, ts(0)], we0[:], dg[:, 0, :],
                                    start=True, stop=True,
                                )
                                nc.tensor.matmul(
                                    psw[:, ts(1)], we1[:, P:], dg[:, 1, :],
                                    start=True, stop=True,
                                )
                                nc.tensor.matmul(
                                    psw[:, ts(2)], we1[:, :P], dg[:, 1, :],
                                    start=True, stop=True,
                                )
                                nc.vector.tensor_copy(
                                    wtp[:, hh, :, :],
                                    psw.rearrange("p (b f) -> p b f", b=3),
                                )
                                yield
                            psa = ps2.tile([P, T], F32, tag="pss")
                            for hh in range(2):
                                h = 2 * j + hh
                                nc.tensor.matmul(
                                    psa[ts(hh, 64), :],
                                    v_sb[:, i, 0, h * 64 : (h + 1) * 64],
                                    wtp[:, hh, 0:2, :].rearrange(
                                        "p b f -> p (b f)"),
                                    start=True, stop=False,
                                )
                                nc.tensor.matmul(
                                    psa[ts(hh, 64), ts(1)],
                                    v_sb[:, i, 1, h * 64 : (h + 1) * 64],
                                    wtp[:, hh, 2, :],
                                    start=False, stop=True,
                                )
                            nc.vector.tensor_copy(attnT[:, j, i, :], psa[:])
                            yield

                    saT = fmp.tile([P, CT, IP, T], BF16, tag="fmA2")
                    for m in range(CT):
                        psj = ps5.tile([P, IP, T], F32, tag="ps5")
                        for k in range(CT):
                            nc.tensor.matmul(
                                psj[:], wp_r[:, k, ts(m)], attnT[:, k, :, :],
                                start=(k == 0), stop=(k == CT - 1),
                            )
                        nc.scalar.activation(
                            saT[:, m, :, :], psj[:], AF.Identity,
                            bias=bpc[:, m : m + 1],
                        )
                        yield
                    x1 = actp.tile([P, IP, TT, C], F32, tag="x1")
                    for i in range(IP):
                        for tt in range(TT):
                            psr = ps5.tile([P, C], BF16, tag="ps5t", bufs=2)
                            for ct in range(CT):
                                nc.tensor.transpose(
                                    psr[:, ts(ct)], saT[:, ct, i, ts(tt)], id_b[:]
                                )
                            nc.vector.tensor_tensor(
                                x1[:, i, tt, :], psr[:], x_t[:, i, tt, :], ALU.add
                            )
                            yield
                    xn2 = yield from ln_stats_gen(x1[:])
                    return x1, xn2

                def back(g, x1, xn2):
                    h2_ct = yield from ln_fm(xn2, g2c, be2c)
                    z = zp.tile([P, FT, IP, T], BF16, tag="z")
                    for m in range(FT):
                        psz = ps5.tile([P, IP, T], F32, tag="ps5")
                        for k in range(CT):
                            nc.tensor.matmul(
                                psz[:], w1_r[:, k, ts(m)], h2_ct[:, k, :, :],
                                start=(k == 0), stop=(k == CT - 1),
                            )
                        if m % 2 == 0:
                            nc.scalar.activation(
                                z[:, m, :, :], psz[:], AF.Relu,
                                bias=b1c[:, m : m + 1],
                            )
                        else:
                            nc.vector.tensor_scalar(
                                z[:, m, :, :], psz[:],
                                b1c[:, m : m + 1], 0.0,
                                ALU.add, ALU.max,
                            )
                        yield
                    yT = fmp.tile([P, CT, IP, T], BF16, tag="fmB2")
                    for m in range(CT):
                        psy = ps5.tile([P, IP, T], F32, tag="ps5")
                        for k in range(FT):
                            nc.tensor.matmul(
                                psy[:], w2_r[:, k, ts(m)], z[:, k, :, :],
                                start=(k == 0), stop=(k == FT - 1),
                            )
                        nc.scalar.activation(
                            yT[:, m, :, :], psy[:], AF.Identity,
                            bias=b2c[:, m : m + 1],
                        )
                        yield
                    for i in range(IP):
                        for tt in range(TT):
                            pso = ps5.tile([P, C], BF16, tag="ps5t", bufs=2)
                            for ct in range(CT):
                                nc.tensor.transpose(
                                    pso[:, ts(ct)], yT[:, ct, i, ts(tt)], id_b[:]
                                )
                            o = actp.tile([P, C], F32, tag="o")
                            nc.vector.tensor_tensor(
                                o[:], pso[:], x1[:, i, tt, :], ALU.add
                            )
                            nc.sync.dma_start(out_t[g * IP + i, ts(tt), :], o[:])
                            yield

                def run_all(gens, strides=None):
                    """Round-robin the generators (gen k advances on rounds
                    divisible by strides[k]); return list of returns."""
                    if strides is None:
                        strides = [1] * len(gens)
                    rets = {}
                    live = {id(gn): gn for gn in gens}
                    order = [(id(gn), st) for gn, st in zip(gens, strides)]
                    rnd = 0
                    while live:
                        for key, st in order:
                            gn = live.get(key)
                            if gn is None or rnd % st != 0:
                                continue
                            try:
                                next(gn)
                            except StopIteration as e:
                                rets[key] = e.value
                                del live[key]
                        rnd += 1
                    return [rets[id(gn)] for gn in gens]

                xs = {0: load_x(0), 1: load_x(1)}
                (xn_cur,) = run_all([ln_stats_gen(xs[0][:])])
                pending = None
                for g in range(NG):
                    if g + 2 < NG:
                        xs[g + 2] = load_x(g + 2)
                    gens = [front(g, xs[g], xn_cur)]
                    strides = [1]
                    if pending is not None:
                        gens.append(back(*pending))
                        strides.append(2)
                    if g + 1 < NG:
                        gens.append(ln_stats_gen(xs[g + 1][:]))
                        strides.append(4)
                    rets = run_all(gens, strides)
                    pending = (g,) + rets[0]
                    xn_cur = rets[-1] if g + 1 < NG else None
                    del xs[g]
                run_all([back(*pending)])
    return nc


_NC_CACHE = None


def _get_nc():
    global _NC_CACHE
    if _NC_CACHE is None:
        _NC_CACHE = build_nc()
    return _NC_CACHE


def _host_consts():
    ident = np.eye(P, dtype=ml_dtypes.bfloat16)
    masks = np.zeros((TT, P, T), dtype=np.float32)
    for tt in range(TT):
        trow = np.arange(P) + tt * P
        scol = np.arange(T)
        masks[tt][scol[None, :] > trow[:, None]] = NEG
    return ident, masks.astype(ml_dtypes.bfloat16)


def kernel(x, Wq, Wk, Wv, Wp, bp, W1, b1, W2, b2, g1, be1, g2, be2):
    bf = ml_dtypes.bfloat16
    x = np.ascontiguousarray(np.asarray(x, np.float32))
    WqF = np.ascontiguousarray(
        np.asarray(Wq, np.float32).transpose(1, 0, 2).reshape(C, C).astype(bf)
    )
    WkF = np.ascontiguousarray(
        np.asarray(Wk, np.float32).transpose(1, 0, 2).reshape(C, C).astype(bf)
    )
    WvF = np.ascontiguousarray(
        np.asarray(Wv, np.float32).transpose(1, 0, 2).reshape(C, C).astype(bf)
    )
    WpF = np.ascontiguousarray(np.asarray(Wp, np.float32).astype(bf))
    W1F = np.ascontiguousarray(np.asarray(W1, np.float32).astype(bf))
    W2F = np.ascontiguousarray(np.asarray(W2, np.float32).astype(bf))
    gb = np.ascontiguousarray(
        np.stack([
            np.asarray(g1, np.float32), np.asarray(be1, np.float32),
            np.asarray(g2, np.float32), np.asarray(be2, np.float32),
            np.asarray(bp, np.float32), np.asarray(b2, np.float32),
        ])
    )
    b1v = np.ascontiguousarray(np.asarray(b1, np.float32))
    ident, masks = _host_consts()

    nc = _get_nc()
    shared = {
        "wqf": WqF, "wkf": WkF, "wvf": WvF, "wpf": WpF,
        "w1f": W1F, "w2f": W2F, "gb": gb, "b1v": b1v,
        "ident": ident, "masks": masks,
    }
    in_maps = []
    for c in range(NCORES):
        m = dict(shared)
        m["x"] = np.ascontiguousarray(x[c * BL : (c + 1) * BL])
        in_maps.append(m)

    from concourse.bass_utils import run_bass_kernel_spmd

    res = run_bass_kernel_spmd(nc, in_maps, list(range(NCORES)))
    out = np.concatenate([res.results[c]["out"] for c in range(NCORES)], axis=0)
    return out.astype(np.float32)
